# revision 31
# baseline (speedup 1.0000x reference)
"""MultiHeadAttention TRN2 Bass kernel, sharded over 8 NeuronCores.

Sharding: 8 cores = 2 batches x 4 head-groups. Each core computes 4 heads of
one batch end-to-end (q/k/v projections, biased+masked softmax attention, and
a partial output projection); the host sums the per-group partial outputs.

On-device layout is fully "transposed" so no on-device transposes are needed:
  - host supplies x^T [D, S] per batch (bf16) and per-core weight slices
  - projections produce qT/kpair [head_dims, S]; v stays natural [S, head_dims]
  - scores are computed transposed: scoresT[s_k, s_q] = kT.T @ qT per head as
    K=64 matmuls (kpair holds the head pair in partition halves; matmul time
    is N-bound so K=64 costs the same as K=128)
  - softmax: exp on ScalarE (PSUM->SBUF), bias/mask applied as a multiply
    with host-precomputed exp(bias_masked)^T on VectorE, and the denominator
    comes free as an extra ones-column in the attn@v matmul
  - attn@v: out2[dh+1, s_q] accumulated over s_k chunks; normalization by the
    ones-row + per-head v-bias correction happens on the way into the concat
    tile; output projection emits partial_out^T [D, S] (bf16) per core.

Phase plan (the exp stream on ScalarE is the hard bottleneck ~1.05us per
[128,1024] tile, 128 tiles):
  - lead-in: all projections, PE-dense, evacs split DVE/ScalarE (both idle)
  - attention: ScalarE saturated with exp; PE/DVE stay under its cadence;
    epilogue (normalize) 3-stage pipelined into the next instance on DVE+DMA
  - tail: outproj sup0 tiles run immediately (they only need sup0 epilogues)
    while the LAST epilogue's DRAM round-trips complete; sup1 tiles follow.
    Evacs on ScalarE first (DVE queue reserved for the epilogue chain), then
    alternating. Output is bf16 to halve the closing DMA.
"""

import numpy as np
import ml_dtypes

import concourse.bass as bass
import concourse.mybir as mybir
import concourse.tile as tile
from concourse.bacc import Bacc

BF16 = mybir.dt.bfloat16
F32 = mybir.dt.float32
nbf16 = ml_dtypes.bfloat16

B = 2
S_FULL = 2048
D = 1024
H = 16
DH = 64
HPC = 4  # heads per core
CD = HPC * DH  # 256 per-core projected dims
NCORES = 8
SCALE = 8.0  # sqrt(DH)

KC = D // 128  # 8 contraction chunks for projections
NB = 512  # projection token-block (free dim per matmul)


def build_module(S=S_FULL, debug=False):
    """Build the single-core Bass program (same program runs SPMD on 8 cores)."""
    assert S % 1024 == 0
    SUPS = 2  # s_q superblocks
    SUPLEN = S // SUPS  # columns per superblock
    NHALF = SUPLEN // NB  # matmuls per psum row-tile
    NT = S // NB  # projection token blocks
    TC = S // 128  # token / s_k chunks

    nc = Bacc(None)

    xqT = nc.dram_tensor("xqT", [D, S], BF16, kind="ExternalInput")
    xkT = nc.dram_tensor("xkT", [D, S], BF16, kind="ExternalInput")
    xvT = nc.dram_tensor("xvT", [D, S], BF16, kind="ExternalInput")
    wqT = nc.dram_tensor("wqT", [128, KC * CD], BF16, kind="ExternalInput")
    wkT = nc.dram_tensor("wkT", [128, KC * CD], BF16, kind="ExternalInput")
    wvT = nc.dram_tensor("wvT", [128, KC * CD], BF16, kind="ExternalInput")
    woT = nc.dram_tensor("woT", [128, (CD // 128) * D], BF16, kind="ExternalInput")
    bqc = nc.dram_tensor("bqc", [128, 2], F32, kind="ExternalInput")
    bkc = nc.dram_tensor("bkc", [128, 2], F32, kind="ExternalInput")
    bvc = nc.dram_tensor("bvc", [64, HPC], F32, kind="ExternalInput")
    expbT = nc.dram_tensor("expbT", [S, S], BF16, kind="ExternalInput")
    poutT = nc.dram_tensor("poutT", [D, S], BF16, kind="ExternalOutput")

    with tile.TileContext(nc) as tc:
        with (
            tc.tile_pool(name="statics", bufs=1) as statics,
            tc.tile_pool(name="xs", bufs=9) as xs_pool,
            tc.tile_pool(name="xv", bufs=KC) as xv_pool,
            tc.tile_pool(name="expb", bufs=2) as expb_pool,
            tc.tile_pool(name="e", bufs=3) as e_pool,
            tc.tile_pool(name="a", bufs=3) as a_pool,
            tc.tile_pool(name="rec", bufs=2) as rec_pool,
            tc.tile_pool(name="spr", bufs=2) as spread_pool,
            tc.tile_pool(name="rb", bufs=2) as rb_pool,
            tc.tile_pool(name="segt", bufs=2) as seg_pool,
            tc.tile_pool(name="oev", bufs=4) as oev_pool,
            tc.tile_pool(name="psc", bufs=2, space="PSUM") as psc,
            tc.tile_pool(name="pacc", bufs=2, space="PSUM") as pacc,
            tc.tile_pool(name="dsc", bufs=4, space="DRAM") as dram_pool,
        ):
            # ---- static tiles ----
            wq_sb = statics.tile([128, KC, CD], BF16, name="wq_sb")
            wk_sb = statics.tile([128, KC, CD], BF16, name="wk_sb")
            wv_sb = statics.tile([128, KC, CD], BF16, name="wv_sb")
            wo_sb = statics.tile([128, CD // 128, D], BF16, name="wo_sb")
            bq_sb = statics.tile([128, 2], F32, name="bq_sb")
            bk_sb = statics.tile([128, 2], F32, name="bk_sb")
            bv_sb = statics.tile([64, HPC], F32, name="bv_sb")
            qT = [statics.tile([128, S], BF16, name=f"qT{m}") for m in range(2)]
            # head-pair kT: head 2m in partitions 0-63, head 2m+1 in 64-127.
            # Scores run as K=64 matmuls (same N-bound cost as padded K=128).
            kpair = [statics.tile([128, S], BF16, name=f"kpair{m}") for m in range(2)]
            vv = statics.tile([128, HPC, TC, DH + 1], BF16, name="vv")
            cc = [statics.tile([128, S], BF16, name=f"cc{m}") for m in range(2)]

            # k weights/inputs first: k-proj gates the first attention instance
            # (wv/wo deferred below so the x streams win the DMA queues)
            nc.sync.dma_start(wk_sb, wkT[:, :].rearrange("p (kc m) -> p kc m", kc=KC))
            nc.sync.dma_start(bk_sb, bkc[:, :])
            nc.sync.dma_start(wq_sb, wqT[:, :].rearrange("p (kc m) -> p kc m", kc=KC))
            nc.sync.dma_start(bq_sb, bqc[:, :])

            nc.gpsimd.memset(vv[:, :, :, DH : DH + 1], 1.0)

            # ---- phase 1: k then q projections (transposed outputs) ----
            # evacs on DVE (tensor_scalar add, per-partition bias); ScalarE is
            # kept free here for the v evacs + warmup so neither queue backs up
            # x loads at [128, 1024] granularity: 256 KB per DMA balances the
            # ~600ns-per-descriptor Sync issue rate against per-queue (~22GB/s)
            # transfer parallelism
            for xdram, w_sb, b_sb, dstt in (
                (xkT, wk_sb, bk_sb, kpair),
                (xqT, wq_sb, bq_sb, qT),
            ):
                for half in range(NT // 2):
                    xts2 = []
                    for kc in range(KC):
                        xt = xs_pool.tile([128, 2 * NB], BF16, name="xt")
                        nc.sync.dma_start(
                            xt,
                            xdram[
                                kc * 128 : (kc + 1) * 128,
                                half * 2 * NB : (half + 1) * 2 * NB,
                            ],
                        )
                        xts2.append(xt)
                    for nt2 in range(2):
                        nt = half * 2 + nt2
                        xts = [t[:, nt2 * NB : (nt2 + 1) * NB] for t in xts2]
                        for mt in range(2):
                            ps = psc.tile([128, NB], F32, name="ps_proj", tag="psc")
                            for kc in range(KC):
                                nc.tensor.matmul(
                                    ps,
                                    lhsT=w_sb[:, kc, mt * 128 : (mt + 1) * 128],
                                    rhs=xts[kc],
                                    start=(kc == 0),
                                    stop=(kc == KC - 1),
                                )
                            csl = slice(nt * NB, (nt + 1) * NB)
                            nc.vector.tensor_scalar_add(
                                dstt[mt][:, csl], ps, scalar1=b_sb[:, mt : mt + 1]
                            )

            # ---- phase 1b: v projection (natural layout, no bias) ----
            # expb sup0 chunks are interleaved between the xv loads: both must
            # land by the end of the lead-in (v-proj consumes xv at the end of
            # the projection stream; the first attention chunks consume expb
            # right after), and neither may delay xk/xq at kernel start.
            nc.sync.dma_start(wv_sb, wvT[:, :].rearrange("p (kc m) -> p kc m", kc=KC))
            nc.sync.dma_start(bv_sb, bvc[:, :])
            expb_tiles = [
                expb_pool.tile([128, TC, SUPLEN], BF16, name="expb")
                for _ in range(SUPS)
            ]
            expb_srcs = [
                expbT[:, sup * SUPLEN : (sup + 1) * SUPLEN].rearrange(
                    "(c p) q -> p c q", p=128
                )
                for sup in range(SUPS)
            ]
            xv_tiles = []
            for kc in range(KC):
                xt = xv_pool.tile([128, S], BF16, name="xvt")
                nc.sync.dma_start(xt, xvT[kc * 128 : (kc + 1) * 128, :])
                xv_tiles.append(xt)
            nc.sync.dma_start(
                wo_sb, woT[:, :].rearrange("p (kc m) -> p kc m", kc=CD // 128)
            )
            for sup in range(SUPS):
                for ckd in range(TC):
                    nc.sync.dma_start(
                        expb_tiles[sup][:, ckd, :], expb_srcs[sup][:, ckd, :]
                    )

            # v evacs alternate ScalarE/DVE (both idle in the lead-in)
            for tk in range(TC):
                ps = pacc.tile([128, CD], F32, name="ps_v", tag="pacc")
                for kc in range(KC):
                    nc.tensor.matmul(
                        ps,
                        lhsT=xv_tiles[kc][:, tk * 128 : (tk + 1) * 128],
                        rhs=wv_sb[:, kc, :],
                        start=(kc == 0),
                        stop=(kc == KC - 1),
                    )
                dst = vv[:, :, tk, 0:DH]
                src = ps.rearrange("p (h d) -> p h d", h=HPC)
                if tk % 2 == 0:
                    nc.scalar.copy(dst, src)
                else:
                    nc.vector.tensor_copy(dst, src)

            # ---- phase 2: attention ----
            # The epilogue (normalize-by-sum) of instance i-1 is software-
            # pipelined into instance i's chunk loop in three stages so the
            # reciprocal/broadcast DMA chain never stalls the in-order DVE
            # stream that feeds PE with A tiles.
            def make_epilogue(sup, h, out2):
                qsl = slice(sup * SUPLEN, (sup + 1) * SUPLEN)
                mt = h // 2
                st = {}

                def s1():
                    # sum row PSUM->SBUF, then spread the 1xN row across 128
                    # partitions via DRAM so the reciprocal runs wide
                    st["ssum"] = rec_pool.tile([DH + 1, SUPLEN], F32, name="ssum")
                    nc.vector.tensor_copy(
                        st["ssum"][DH : DH + 1, :], out2[DH : DH + 1, :]
                    )
                    st["rsd"] = dram_pool.tile([1, SUPLEN], F32, name="rsd")
                    nc.sync.dma_start(st["rsd"], st["ssum"][DH : DH + 1, :])
                    st["spread"] = spread_pool.tile([128, SUPLEN // 128], F32, name="spread")
                    nc.sync.dma_start(
                        st["spread"],
                        st["rsd"][:, :].rearrange("a (p f) -> (a p) f", p=128),
                    )

                def s2():
                    nc.vector.reciprocal(st["spread"], st["spread"])
                    st["rsd2"] = dram_pool.tile([1, SUPLEN], F32, name="rsd2")
                    nc.sync.dma_start(
                        st["rsd2"][:, :].rearrange("a (p f) -> (a p) f", p=128),
                        st["spread"],
                    )
                    st["rb"] = rb_pool.tile([64, SUPLEN], F32, name="rb")
                    nc.sync.dma_start(
                        st["rb"], st["rsd2"][:, :].partition_broadcast(64)
                    )

                def s3():
                    rb = st["rb"]
                    if h % 2 == 0:
                        seg = cc[mt][0:64, qsl]
                        nc.vector.tensor_mul(seg, out2[0:DH, :], rb)
                        nc.vector.tensor_scalar_add(
                            seg, seg, scalar1=bv_sb[:, h : h + 1]
                        )
                    else:
                        segt = seg_pool.tile([64, SUPLEN], BF16, name="segt")
                        nc.vector.tensor_mul(segt, out2[0:DH, :], rb)
                        nc.vector.tensor_scalar_add(
                            segt, segt, scalar1=bv_sb[:, h : h + 1]
                        )
                        # partition move 0-63 -> 64-127 via DMA
                        nc.sync.dma_start(cc[mt][64:128, qsl], segt)

                return (s1, s2, s3)

            # The scores matmuls are emitted TWO chunks ahead of the attn@v
            # matmuls of the same chunk. The PE queue is in-order, and av(ck)
            # waits on the exp->mul chain of ck; with sc(ck+2) ahead of av(ck)
            # in the queue, the exp stream never waits on a scores matmul that
            # is stuck behind a data-dependent attn@v (this ordering is what
            # keeps the ScalarE exp cadence back-to-back).
            def emit_scores(sup, mt, hh, ck, sc):
                lhsT_k = kpair[mt][hh * 64 : (hh + 1) * 64, ck * 128 : (ck + 1) * 128]
                for hf in range(NHALF):
                    nc.tensor.matmul(
                        sc[:, hf * NB : (hf + 1) * NB],
                        lhsT=lhsT_k,
                        rhs=qT[mt][
                            hh * 64 : (hh + 1) * 64,
                            sup * SUPLEN + hf * NB : sup * SUPLEN + (hf + 1) * NB,
                        ],
                        start=True,
                        stop=True,
                    )

            pending = None
            for sup in range(SUPS):
                for h in range(HPC):
                    mt = h // 2
                    hh = h % 2
                    scs = {}
                    for ck in (0, 1):
                        scs[ck] = psc.tile([128, SUPLEN], F32, name="sc", tag="psc")
                        emit_scores(sup, mt, hh, ck, scs[ck])
                    out2 = pacc.tile([DH + 1, SUPLEN], F32, name="out2", tag="pacc")
                    for ck in range(TC):
                        sc = scs.pop(ck)
                        e = e_pool.tile([128, SUPLEN], BF16, name="e")
                        nc.scalar.activation(
                            e, sc, func=mybir.ActivationFunctionType.Exp
                        )
                        a = a_pool.tile([128, SUPLEN], BF16, name="a")
                        nc.vector.tensor_mul(a, e, expb_tiles[sup][:, ck, :])
                        if ck + 2 < TC:
                            scs[ck + 2] = psc.tile(
                                [128, SUPLEN], F32, name="sc", tag="psc"
                            )
                            emit_scores(sup, mt, hh, ck + 2, scs[ck + 2])
                        for hf in range(NHALF):
                            hsl = slice(hf * NB, (hf + 1) * NB)
                            nc.tensor.matmul(
                                out2[:, hsl],
                                lhsT=vv[:, h, ck, :],
                                rhs=a[:, hsl],
                                start=(ck == 0),
                                stop=(ck == TC - 1),
                            )
                        if pending is not None:
                            if ck == 0:
                                pending[0]()
                            elif ck == TC // 4:
                                pending[1]()
                            elif ck == TC // 2:
                                pending[2]()
                    pending = make_epilogue(sup, h, out2)

            # ---- phase 3: output projection tail ----
            # sup-0 tiles first: they depend only on sup-0 epilogues (already
            # done), so PE stays dense while the LAST epilogue's DMA chain
            # (interleaved below) completes; sup-1 tiles follow. Evacs go to
            # ScalarE first (DVE queue is reserved for the epilogue stages so
            # neither blocks the other), then alternate for rate.
            # The tail was paced ~1.4us per output tile by the per-tile
            # mm->evac->dma-issue handshakes, invariant to pool depth. So emit
            # HALF as many units, each twice as wide: [128, 2*NB] psum (same
            # 2-bank footprint as one sc buffer), one evacuation and one DMA
            # per 256 KB instead of per 128 KB.
            op_serial = [0]

            def outproj_tile(mo, ntp, evac_engine, psc_only=False):
                i = op_serial[0]
                op_serial[0] += 1
                if psc_only:
                    pool, tag = psc, "psc"
                else:
                    pool, tag = (psc, "psc") if i % 2 == 0 else (pacc, "pacc")
                ps = pool.tile([128, 2, NB], F32, name="ps_o", tag=tag)
                for kc in range(CD // 128):
                    for nh in range(2):
                        nt = 2 * ntp + nh
                        nc.tensor.matmul(
                            ps[:, nh, :],
                            lhsT=wo_sb[:, kc, mo * 128 : (mo + 1) * 128],
                            rhs=cc[kc][:, nt * NB : (nt + 1) * NB],
                            start=(kc == 0),
                            stop=(kc == CD // 128 - 1),
                        )
                ot = oev_pool.tile([128, 2 * NB], BF16, name="ot")
                src = ps.rearrange("p a b -> p (a b)")
                if evac_engine == "scalar":
                    nc.scalar.copy(ot, src)
                else:
                    nc.vector.tensor_copy(ot, src)
                nc.sync.dma_start(
                    poutT[
                        mo * 128 : (mo + 1) * 128,
                        ntp * 2 * NB : (ntp + 1) * 2 * NB,
                    ],
                    ot,
                )

            # Last epilogue (sup1,h3) gates ALL sup1 columns; its stages go out
            # first so the DMA chain runs while the sup0 tiles (which need only
            # the long-done sup0 epilogues) keep PE busy. Those early tiles use
            # the psc pool only and evacuate on ScalarE, so nothing on PE or
            # DVE ever queues behind the epilogue's semaphore waits or touches
            # the pacc bank the final s3 frees.
            tail_units = [(mo, 0) for mo in range(D // 128)]  # sup0 columns
            tail_units += [(mo, 1) for mo in range(D // 128)]  # sup1 columns
            emitted = 0
            stages_done = 0
            for mo, ntp in tail_units:
                psc_only = stages_done < 3 or emitted < 4
                outproj_tile(
                    mo, ntp,
                    "scalar" if emitted < 4 else ("vector" if emitted % 2 else "scalar"),
                    psc_only=psc_only,
                )
                emitted += 1
                if pending is not None and stages_done < 3:
                    pending[stages_done]()
                    stages_done += 1
            if pending is not None:
                while stages_done < 3:
                    pending[stages_done]()
                    stages_done += 1
                pending = None

    nc.finalize()
    return nc


def make_in_maps(query, key, value, mask, chemical_bias, Wq, bq, Wk, bk, Wv, bv, Wo, S=S_FULL):
    """Host-side preprocessing: per-core input dicts (8 cores)."""
    f32 = np.float32

    def c(a, dt):
        return np.ascontiguousarray(a, dtype=dt)

    per_batch = []
    for b in range(B):
        xq = c(query[b].T, nbf16)
        xk = c(key[b].T, nbf16)
        xv = c(value[b].T, nbf16)
        bm = np.where(mask[b, 0] == 0, f32(0.0), np.exp(chemical_bias[b], dtype=f32))
        expbT_ = c(bm.T, nbf16)
        per_batch.append((xq, xk, xv, expbT_))

    def warr(wt, kc):
        # [kc*128, M] -> [128, kc*M]: per-partition-contiguous device layout
        m = wt.shape[1]
        return np.ascontiguousarray(
            wt.reshape(kc, 128, m).transpose(1, 0, 2).reshape(128, kc * m), nbf16
        )

    per_group = []
    for g in range(4):
        hsl = slice(g * CD, (g + 1) * CD)
        wqT_ = warr(np.asarray((Wq[hsl] / SCALE).T, np.float32), KC)
        wkT_ = warr(np.asarray(Wk[hsl].T, np.float32), KC)
        wvT_ = warr(np.asarray(Wv[hsl].T, np.float32), KC)
        woT_ = warr(np.asarray(Wo[:, hsl].T, np.float32), CD // 128)
        bqc_ = c((bq[hsl] / SCALE).reshape(2, 128).T, f32)
        bkc_ = c(bk[hsl].reshape(2, 128).T, f32)
        bvc_ = c(bv[hsl].reshape(HPC, 64).T, f32)
        per_group.append((wqT_, wkT_, wvT_, woT_, bqc_, bkc_, bvc_))

    in_maps = []
    for core in range(NCORES):
        b, g = divmod(core, 4)
        xq, xk, xv, expbT_ = per_batch[b]
        wqT_, wkT_, wvT_, woT_, bqc_, bkc_, bvc_ = per_group[g]
        in_maps.append(
            {
                "xqT": xq,
                "xkT": xk,
                "xvT": xv,
                "wqT": wqT_,
                "wkT": wkT_,
                "wvT": wvT_,
                "woT": woT_,
                "bqc": bqc_,
                "bkc": bkc_,
                "bvc": bvc_,
                "expbT": expbT_,
            }
        )
    return in_maps


def combine_outputs(results, bo):
    """Sum per-group transposed partials into the full [B, S, D] output."""
    out = np.empty((B, S_FULL, D), np.float32)
    for b in range(B):
        acc = results[4 * b]["poutT"].T.astype(np.float32)
        for g in range(1, 4):
            acc = acc + results[4 * b + g]["poutT"].T.astype(np.float32)
        out[b] = acc + bo.astype(np.float32)
    return out


_NC_CACHE = {}


def _get_module(S=S_FULL, debug=False):
    key = (S, debug)
    if key not in _NC_CACHE:
        _NC_CACHE[key] = build_module(S, debug=debug)
    return _NC_CACHE[key]


def run_spmd(in_maps, S=S_FULL, debug=False, **kwargs):
    from concourse.bass_utils import run_bass_kernel_spmd

    nc = _get_module(S, debug)
    return run_bass_kernel_spmd(nc, in_maps, core_ids=list(range(NCORES)), **kwargs)


def kernel(query, key, value, mask, chemical_bias, Wq, bq, Wk, bk, Wv, bv, Wo, bo):
    in_maps = make_in_maps(
        query, key, value, mask, chemical_bias, Wq, bq, Wk, bk, Wv, bv, Wo
    )
    res = run_spmd(in_maps)
    return combine_outputs(res.results, bo)


# revision 35
# speedup vs baseline: 1.0585x; 1.0585x over previous
"""MultiHeadAttention TRN2 Bass kernel, sharded over 8 NeuronCores.

Sharding: 8 cores = 2 batches x 4 head-groups. Each core computes 4 heads of
one batch end-to-end (q/k/v projections, biased+masked softmax attention, and
a partial output projection); the host sums the per-group partial outputs.

On-device layout is fully "transposed" so no on-device transposes are needed:
  - host supplies x^T [D, S] per batch (bf16) and per-core weight slices
  - projections produce qT/kpair [head_dims, S]; v stays natural [S, head_dims]
  - scores are computed transposed: scoresT[s_k, s_q] = kT.T @ qT per head as
    K=64 matmuls (kpair holds the head pair in partition halves; matmul time
    is N-bound so K=64 costs the same as K=128)
  - softmax: exp on ScalarE (PSUM->SBUF), bias/mask applied as a multiply
    with host-precomputed exp(bias_masked)^T on VectorE, and the denominator
    comes free as an extra ones-column in the attn@v matmul
  - attn@v: out2[dh+1, s_q] accumulated over s_k chunks; normalization by the
    ones-row + per-head v-bias correction happens on the way into the concat
    tile; output projection emits partial_out^T [D, S] (bf16) per core.

Phase plan (the exp stream on ScalarE is the hard bottleneck ~1.05us per
[128,1024] tile, 128 tiles):
  - lead-in: all projections, PE-dense, evacs split DVE/ScalarE (both idle)
  - attention: ScalarE saturated with exp; PE/DVE stay under its cadence;
    epilogue (normalize) 3-stage pipelined into the next instance on DVE+DMA
  - tail: outproj sup0 tiles run immediately (they only need sup0 epilogues)
    while the LAST epilogue's DRAM round-trips complete; sup1 tiles follow.
    Evacs on ScalarE first (DVE queue reserved for the epilogue chain), then
    alternating. Output is bf16 to halve the closing DMA.
"""

import numpy as np
import ml_dtypes

import concourse.bass as bass
import concourse.mybir as mybir
import concourse.tile as tile
from concourse.bacc import Bacc

BF16 = mybir.dt.bfloat16
F32 = mybir.dt.float32
nbf16 = ml_dtypes.bfloat16

B = 2
S_FULL = 2048
D = 1024
H = 16
DH = 64
HPC = 4  # heads per core
CD = HPC * DH  # 256 per-core projected dims
NCORES = 8
SCALE = 8.0  # sqrt(DH)

KC = D // 128  # 8 contraction chunks for projections
NB = 512  # projection token-block (free dim per matmul)


def build_module(S=S_FULL, debug=False):
    """Build the single-core Bass program (same program runs SPMD on 8 cores)."""
    assert S % 1024 == 0
    SUPS = 2  # s_q superblocks
    SUPLEN = S // SUPS  # columns per superblock
    NHALF = SUPLEN // NB  # matmuls per psum row-tile
    NT = S // NB  # projection token blocks
    TC = S // 128  # token / s_k chunks

    nc = Bacc(None)

    xqT = nc.dram_tensor("xqT", [D, S], BF16, kind="ExternalInput")
    xkT = nc.dram_tensor("xkT", [D, S], BF16, kind="ExternalInput")
    xvT = nc.dram_tensor("xvT", [D, S], BF16, kind="ExternalInput")
    wqT = nc.dram_tensor("wqT", [128, KC * CD], BF16, kind="ExternalInput")
    wkT = nc.dram_tensor("wkT", [128, KC * CD], BF16, kind="ExternalInput")
    wvT = nc.dram_tensor("wvT", [128, KC * CD], BF16, kind="ExternalInput")
    woT = nc.dram_tensor("woT", [128, (CD // 128) * D], BF16, kind="ExternalInput")
    bqc = nc.dram_tensor("bqc", [128, 2], F32, kind="ExternalInput")
    bkc = nc.dram_tensor("bkc", [128, 2], F32, kind="ExternalInput")
    bvc = nc.dram_tensor("bvc", [64, HPC], F32, kind="ExternalInput")
    expbT = nc.dram_tensor("expbT", [S, S], BF16, kind="ExternalInput")
    poutT = nc.dram_tensor("poutT", [D, S], BF16, kind="ExternalOutput")

    with tile.TileContext(nc) as tc:
        with (
            tc.tile_pool(name="statics", bufs=1) as statics,
            tc.tile_pool(name="xs", bufs=10) as xs_pool,
            tc.tile_pool(name="xv", bufs=KC) as xv_pool,
            tc.tile_pool(name="expb", bufs=2) as expb_pool,
            tc.tile_pool(name="e", bufs=4) as e_pool,
            tc.tile_pool(name="a", bufs=4) as a_pool,
            tc.tile_pool(name="rec", bufs=2) as rec_pool,
            tc.tile_pool(name="spr", bufs=2) as spread_pool,
            tc.tile_pool(name="rb", bufs=2) as rb_pool,
            tc.tile_pool(name="segt", bufs=2) as seg_pool,
            tc.tile_pool(name="oev", bufs=3) as oev_pool,
            tc.tile_pool(name="psc", bufs=2, space="PSUM") as psc,
            tc.tile_pool(name="pacc", bufs=2, space="PSUM") as pacc,
            tc.tile_pool(name="dsc", bufs=4, space="DRAM") as dram_pool,
        ):
            # ---- static tiles ----
            wq_sb = statics.tile([128, KC, CD], BF16, name="wq_sb")
            wk_sb = statics.tile([128, KC, CD], BF16, name="wk_sb")
            wv_sb = statics.tile([128, KC, CD], BF16, name="wv_sb")
            wo_sb = statics.tile([128, CD // 128, D], BF16, name="wo_sb")
            bq_sb = statics.tile([128, 2], F32, name="bq_sb")
            bk_sb = statics.tile([128, 2], F32, name="bk_sb")
            bv_sb = statics.tile([64, HPC], F32, name="bv_sb")
            qT = [statics.tile([128, S], BF16, name=f"qT{m}") for m in range(2)]
            # head-pair kT: head 2m in partitions 0-63, head 2m+1 in 64-127.
            # Scores run as K=64 matmuls (same N-bound cost as padded K=128).
            kpair = [statics.tile([128, S], BF16, name=f"kpair{m}") for m in range(2)]
            vv = statics.tile([128, HPC, TC, DH + 1], BF16, name="vv")
            cc = [statics.tile([128, S], BF16, name=f"cc{m}") for m in range(2)]

            # k weights/inputs first: k-proj gates the first attention instance
            # (wv/wo deferred below so the x streams win the DMA queues)
            nc.sync.dma_start(wk_sb, wkT[:, :].rearrange("p (kc m) -> p kc m", kc=KC))
            nc.sync.dma_start(bk_sb, bkc[:, :])

            nc.gpsimd.memset(vv[:, :, :, DH : DH + 1], 1.0)

            # ---- phase 1: k then q projections (transposed outputs) ----
            # evacs on DVE (tensor_scalar add, per-partition bias); ScalarE is
            # kept free here for the v evacs + warmup so neither queue backs up
            # x loads at [128, 1024] granularity: 256 KB per DMA balances the
            # ~600ns-per-descriptor Sync issue rate against per-queue (~22GB/s)
            # transfer parallelism
            for xdram, w_sb, b_sb, dstt in (
                (xkT, wk_sb, bk_sb, kpair),
                (xqT, wq_sb, bq_sb, qT),
            ):
                if xdram is xqT:
                    # wq queued after the xk stream: k-proj (which gates the
                    # first attention instance) starts a little sooner, and wq
                    # still lands well before the first q-proj matmul
                    nc.sync.dma_start(
                        wq_sb, wqT[:, :].rearrange("p (kc m) -> p kc m", kc=KC)
                    )
                    nc.sync.dma_start(bq_sb, bqc[:, :])
                for half in range(NT // 2):
                    xts2 = []
                    for kc in range(KC):
                        xt = xs_pool.tile([128, 2 * NB], BF16, name="xt")
                        nc.sync.dma_start(
                            xt,
                            xdram[
                                kc * 128 : (kc + 1) * 128,
                                half * 2 * NB : (half + 1) * 2 * NB,
                            ],
                        )
                        xts2.append(xt)
                    for nt2 in range(2):
                        nt = half * 2 + nt2
                        xts = [t[:, nt2 * NB : (nt2 + 1) * NB] for t in xts2]
                        for mt in range(2):
                            ps = psc.tile([128, NB], F32, name="ps_proj", tag="psc")
                            for kc in range(KC):
                                nc.tensor.matmul(
                                    ps,
                                    lhsT=w_sb[:, kc, mt * 128 : (mt + 1) * 128],
                                    rhs=xts[kc],
                                    start=(kc == 0),
                                    stop=(kc == KC - 1),
                                )
                            csl = slice(nt * NB, (nt + 1) * NB)
                            nc.vector.tensor_scalar_add(
                                dstt[mt][:, csl], ps, scalar1=b_sb[:, mt : mt + 1]
                            )

            # ---- phase 1b: v projection (natural layout, no bias) ----
            # expb sup0 chunks are interleaved between the xv loads: both must
            # land by the end of the lead-in (v-proj consumes xv at the end of
            # the projection stream; the first attention chunks consume expb
            # right after), and neither may delay xk/xq at kernel start.
            nc.sync.dma_start(wv_sb, wvT[:, :].rearrange("p (kc m) -> p kc m", kc=KC))
            nc.sync.dma_start(bv_sb, bvc[:, :])
            expb_tiles = [
                expb_pool.tile([128, TC, SUPLEN], BF16, name="expb")
                for _ in range(SUPS)
            ]
            expb_srcs = [
                expbT[:, sup * SUPLEN : (sup + 1) * SUPLEN].rearrange(
                    "(c p) q -> p c q", p=128
                )
                for sup in range(SUPS)
            ]
            xv_tiles = []
            for kc in range(KC):
                xt = xv_pool.tile([128, S], BF16, name="xvt")
                nc.sync.dma_start(xt, xvT[kc * 128 : (kc + 1) * 128, :])
                xv_tiles.append(xt)
            nc.sync.dma_start(
                wo_sb, woT[:, :].rearrange("p (kc m) -> p kc m", kc=CD // 128)
            )
            for sup in range(SUPS):
                for ckd in range(TC):
                    nc.sync.dma_start(
                        expb_tiles[sup][:, ckd, :], expb_srcs[sup][:, ckd, :]
                    )

            # v evacs alternate ScalarE/DVE (both idle in the lead-in)
            for tk in range(TC):
                ps = pacc.tile([128, CD], F32, name="ps_v", tag="pacc")
                for kc in range(KC):
                    nc.tensor.matmul(
                        ps,
                        lhsT=xv_tiles[kc][:, tk * 128 : (tk + 1) * 128],
                        rhs=wv_sb[:, kc, :],
                        start=(kc == 0),
                        stop=(kc == KC - 1),
                    )
                dst = vv[:, :, tk, 0:DH]
                src = ps.rearrange("p (h d) -> p h d", h=HPC)
                if tk % 2 == 0:
                    nc.scalar.copy(dst, src)
                else:
                    nc.vector.tensor_copy(dst, src)

            # ---- phase 2: attention ----
            # The epilogue (normalize-by-sum) of instance i-1 is software-
            # pipelined into instance i's chunk loop in three stages so the
            # reciprocal/broadcast DMA chain never stalls the in-order DVE
            # stream that feeds PE with A tiles.
            def make_epilogue(sup, h, out2):
                qsl = slice(sup * SUPLEN, (sup + 1) * SUPLEN)
                mt = h // 2
                st = {}

                def s1():
                    # sum row PSUM->SBUF, then spread the 1xN row across 128
                    # partitions via DRAM so the reciprocal runs wide
                    st["ssum"] = rec_pool.tile([DH + 1, SUPLEN], F32, name="ssum")
                    nc.vector.tensor_copy(
                        st["ssum"][DH : DH + 1, :], out2[DH : DH + 1, :]
                    )
                    st["rsd"] = dram_pool.tile([1, SUPLEN], F32, name="rsd")
                    nc.sync.dma_start(st["rsd"], st["ssum"][DH : DH + 1, :])
                    st["spread"] = spread_pool.tile([128, SUPLEN // 128], F32, name="spread")
                    nc.sync.dma_start(
                        st["spread"],
                        st["rsd"][:, :].rearrange("a (p f) -> (a p) f", p=128),
                    )

                def s2():
                    nc.vector.reciprocal(st["spread"], st["spread"])
                    st["rsd2"] = dram_pool.tile([1, SUPLEN], F32, name="rsd2")
                    nc.sync.dma_start(
                        st["rsd2"][:, :].rearrange("a (p f) -> (a p) f", p=128),
                        st["spread"],
                    )
                    st["rb"] = rb_pool.tile([64, SUPLEN], F32, name="rb")
                    nc.sync.dma_start(
                        st["rb"], st["rsd2"][:, :].partition_broadcast(64)
                    )

                def s3():
                    rb = st["rb"]
                    if h % 2 == 0:
                        seg = cc[mt][0:64, qsl]
                        nc.vector.tensor_mul(seg, out2[0:DH, :], rb)
                        nc.vector.tensor_scalar_add(
                            seg, seg, scalar1=bv_sb[:, h : h + 1]
                        )
                    else:
                        segt = seg_pool.tile([64, SUPLEN], BF16, name="segt")
                        nc.vector.tensor_mul(segt, out2[0:DH, :], rb)
                        nc.vector.tensor_scalar_add(
                            segt, segt, scalar1=bv_sb[:, h : h + 1]
                        )
                        # partition move 0-63 -> 64-127 via DMA
                        nc.sync.dma_start(cc[mt][64:128, qsl], segt)

                return (s1, s2, s3)

            # The scores matmuls are emitted TWO chunks ahead of the attn@v
            # matmuls of the same chunk. The PE queue is in-order, and av(ck)
            # waits on the exp->mul chain of ck; with sc(ck+2) ahead of av(ck)
            # in the queue, the exp stream never waits on a scores matmul that
            # is stuck behind a data-dependent attn@v (this ordering is what
            # keeps the ScalarE exp cadence back-to-back).
            def emit_scores(sup, mt, hh, ck, sc):
                lhsT_k = kpair[mt][hh * 64 : (hh + 1) * 64, ck * 128 : (ck + 1) * 128]
                for hf in range(NHALF):
                    nc.tensor.matmul(
                        sc[:, hf * NB : (hf + 1) * NB],
                        lhsT=lhsT_k,
                        rhs=qT[mt][
                            hh * 64 : (hh + 1) * 64,
                            sup * SUPLEN + hf * NB : sup * SUPLEN + (hf + 1) * NB,
                        ],
                        start=True,
                        stop=True,
                    )

            pending = None
            for sup in range(SUPS):
                for h in range(HPC):
                    mt = h // 2
                    hh = h % 2
                    scs = {}
                    for ck in (0, 1):
                        scs[ck] = psc.tile([128, SUPLEN], F32, name="sc", tag="psc")
                        emit_scores(sup, mt, hh, ck, scs[ck])
                    out2 = pacc.tile([DH + 1, SUPLEN], F32, name="out2", tag="pacc")
                    for ck in range(TC):
                        sc = scs.pop(ck)
                        e = e_pool.tile([128, SUPLEN], BF16, name="e")
                        nc.scalar.activation(
                            e, sc, func=mybir.ActivationFunctionType.Exp
                        )
                        a = a_pool.tile([128, SUPLEN], BF16, name="a")
                        nc.vector.tensor_mul(a, e, expb_tiles[sup][:, ck, :])
                        if ck + 2 < TC:
                            scs[ck + 2] = psc.tile(
                                [128, SUPLEN], F32, name="sc", tag="psc"
                            )
                            emit_scores(sup, mt, hh, ck + 2, scs[ck + 2])
                        for hf in range(NHALF):
                            hsl = slice(hf * NB, (hf + 1) * NB)
                            nc.tensor.matmul(
                                out2[:, hsl],
                                lhsT=vv[:, h, ck, :],
                                rhs=a[:, hsl],
                                start=(ck == 0),
                                stop=(ck == TC - 1),
                            )
                        if pending is not None:
                            if ck == 0:
                                pending[0]()
                            elif ck == TC // 4:
                                pending[1]()
                            elif ck == TC // 2:
                                pending[2]()
                    pending = make_epilogue(sup, h, out2)

            # ---- phase 3: output projection tail ----
            # sup-0 tiles first: they depend only on sup-0 epilogues (already
            # done), so PE stays dense while the LAST epilogue's DMA chain
            # (interleaved below) completes; sup-1 tiles follow. Evacs go to
            # ScalarE first (DVE queue is reserved for the epilogue stages so
            # neither blocks the other), then alternate for rate.
            op_serial = [0]

            def outproj_tile(mo, nt, evac_engine, psc_only=False):
                i = op_serial[0]
                op_serial[0] += 1
                if psc_only:
                    pool, tag = psc, "psc"
                else:
                    pool, tag = (psc, "psc") if i % 2 == 0 else (pacc, "pacc")
                ps = pool.tile([128, NB], F32, name="ps_o", tag=tag)
                for kc in range(CD // 128):
                    nc.tensor.matmul(
                        ps,
                        lhsT=wo_sb[:, kc, mo * 128 : (mo + 1) * 128],
                        rhs=cc[kc][:, nt * NB : (nt + 1) * NB],
                        start=(kc == 0),
                        stop=(kc == CD // 128 - 1),
                    )
                ot = oev_pool.tile([128, NB], BF16, name="ot")
                if evac_engine == "scalar":
                    nc.scalar.copy(ot, ps)
                else:
                    nc.vector.tensor_copy(ot, ps)
                nc.sync.dma_start(
                    poutT[mo * 128 : (mo + 1) * 128, nt * NB : (nt + 1) * NB], ot
                )

            # Last epilogue (sup1,h3) gates ALL sup1 columns; its stages go out
            # first so the DMA chain runs while the sup0 tiles (which need only
            # the long-done sup0 epilogues) keep PE busy. Those early tiles use
            # the psc pool only and evacuate on ScalarE, so nothing on PE or
            # DVE ever queues behind the epilogue's semaphore waits or touches
            # the pacc bank the final s3 frees.
            tail_tiles = [(mo, nt) for nt in range(NT // 2) for mo in range(D // 128)]
            tail_tiles += [(mo, nt) for nt in range(NT // 2, NT) for mo in range(D // 128)]
            emitted = 0
            stages_done = 0
            for mo, nt in tail_tiles:
                psc_only = stages_done < 3 or emitted < 8
                outproj_tile(
                    mo, nt,
                    "scalar" if emitted < 8 else ("vector" if emitted % 2 else "scalar"),
                    psc_only=psc_only,
                )
                emitted += 1
                if pending is not None and stages_done < 3:
                    pending[stages_done]()
                    stages_done += 1
            if pending is not None:
                while stages_done < 3:
                    pending[stages_done]()
                    stages_done += 1
                pending = None

    nc.finalize()
    return nc


def make_in_maps(query, key, value, mask, chemical_bias, Wq, bq, Wk, bk, Wv, bv, Wo, S=S_FULL):
    """Host-side preprocessing: per-core input dicts (8 cores)."""
    f32 = np.float32

    def c(a, dt):
        return np.ascontiguousarray(a, dtype=dt)

    per_batch = []
    for b in range(B):
        xq = c(query[b].T, nbf16)
        xk = c(key[b].T, nbf16)
        xv = c(value[b].T, nbf16)
        bm = np.where(mask[b, 0] == 0, f32(0.0), np.exp(chemical_bias[b], dtype=f32))
        expbT_ = c(bm.T, nbf16)
        per_batch.append((xq, xk, xv, expbT_))

    def warr(wt, kc):
        # [kc*128, M] -> [128, kc*M]: per-partition-contiguous device layout
        m = wt.shape[1]
        return np.ascontiguousarray(
            wt.reshape(kc, 128, m).transpose(1, 0, 2).reshape(128, kc * m), nbf16
        )

    per_group = []
    for g in range(4):
        hsl = slice(g * CD, (g + 1) * CD)
        wqT_ = warr(np.asarray((Wq[hsl] / SCALE).T, np.float32), KC)
        wkT_ = warr(np.asarray(Wk[hsl].T, np.float32), KC)
        wvT_ = warr(np.asarray(Wv[hsl].T, np.float32), KC)
        woT_ = warr(np.asarray(Wo[:, hsl].T, np.float32), CD // 128)
        bqc_ = c((bq[hsl] / SCALE).reshape(2, 128).T, f32)
        bkc_ = c(bk[hsl].reshape(2, 128).T, f32)
        bvc_ = c(bv[hsl].reshape(HPC, 64).T, f32)
        per_group.append((wqT_, wkT_, wvT_, woT_, bqc_, bkc_, bvc_))

    in_maps = []
    for core in range(NCORES):
        b, g = divmod(core, 4)
        xq, xk, xv, expbT_ = per_batch[b]
        wqT_, wkT_, wvT_, woT_, bqc_, bkc_, bvc_ = per_group[g]
        in_maps.append(
            {
                "xqT": xq,
                "xkT": xk,
                "xvT": xv,
                "wqT": wqT_,
                "wkT": wkT_,
                "wvT": wvT_,
                "woT": woT_,
                "bqc": bqc_,
                "bkc": bkc_,
                "bvc": bvc_,
                "expbT": expbT_,
            }
        )
    return in_maps


def combine_outputs(results, bo):
    """Sum per-group transposed partials into the full [B, S, D] output."""
    out = np.empty((B, S_FULL, D), np.float32)
    for b in range(B):
        acc = results[4 * b]["poutT"].T.astype(np.float32)
        for g in range(1, 4):
            acc = acc + results[4 * b + g]["poutT"].T.astype(np.float32)
        out[b] = acc + bo.astype(np.float32)
    return out


_NC_CACHE = {}


def _get_module(S=S_FULL, debug=False):
    key = (S, debug)
    if key not in _NC_CACHE:
        _NC_CACHE[key] = build_module(S, debug=debug)
    return _NC_CACHE[key]


def run_spmd(in_maps, S=S_FULL, debug=False, **kwargs):
    from concourse.bass_utils import run_bass_kernel_spmd

    nc = _get_module(S, debug)
    return run_bass_kernel_spmd(nc, in_maps, core_ids=list(range(NCORES)), **kwargs)


def kernel(query, key, value, mask, chemical_bias, Wq, bq, Wk, bk, Wv, bv, Wo, bo):
    in_maps = make_in_maps(
        query, key, value, mask, chemical_bias, Wq, bq, Wk, bk, Wv, bv, Wo
    )
    res = run_spmd(in_maps)
    return combine_outputs(res.results, bo)
